# revision 4
# baseline (speedup 1.0000x reference)
"""Trainium2 Bass kernel for nn_LossFunction_12532714569881.

Computes, for x: [N=8192, 2, D=256] fp32, w, b scalars:
    P = x[:,0,:]; A = x[:,1,:]
    logits = (P @ A^T) / max(|p_i||a_j|, eps) * w + b        # [N, N]
    loss = -mean_i(log_softmax(logits)[i, i])

Strategy (8 NeuronCores, SPMD, single launch):
  - Row-shard the NxN logits: core c owns rows R=c*1024 .. R+1024.
  - Each core receives its positive block (xp) and the FULL anchor
    matrix ROTATED so its own 1024 anchors come first (xa_rot); the
    diagonal block is then always tiles 0..7 of group 0 -- one NEFF
    works for all cores and the separate diag-anchor load is gone.
  - Loads are PRIORITIZED: xp + anchor group 0 stream on the ACT hwdge
    queue set (issued first, in halves for early compute start); groups
    1-3 stream behind them on the gpsimd swdge queue set.  Compute on
    group g overlaps the loads of groups g+1..
  - Anchors: per-group sum-of-squares on GpSimd (otherwise idle),
    1/norm via exp(-0.5*ln(s)) on ACT (single activation table set via
    _patch_act_tables), normalize+bf16 cast on DVE, then transposed
    SBUF->SBUF over the DMA xbar (2-byte path, 128x128 contiguous
    blocks, issued on the sync hwdge queues) -- the tensor engine and
    PSUM never touch transposes, so matmul PSUM ping-pong is never
    interrupted at group boundaries.
  - Positives stay raw: the per-row scale w/|p_i| folds into the exp
    activation's per-partition scale operand.
  - Main loop per column group: bf16 matmuls accumulate K=256 in two
    128-chunks into [128, 2048] PSUM tiles (2 bufs x 4 banks); the
    scalar engine applies exp(scale_i * dot - |w|) with a fused row-sum
    (accum_out).  Group 0 is processed in two 1024-column halves so the
    exp stream starts as soon as the first 1MB of anchors lands.
  - Since cos in [-1,1], logits <= |w|+b, so the constant shift |w|+b
    replaces the row-max pass of a standard softmax (no overflow).
  - The diagonal logit (the label term) is recomputed exactly in fp32
    on the vector engine from the raw blocks, so the bf16 matmul noise
    only perturbs the log-sum-exp, where it averages out.
  - Each core emits one partial scalar = sum of its 1024 row losses
    (row loss = ln(S_i) + |w| - w*cos_ii); the host sums 8 partials,
    divides by N.

kernel(**inputs) -> np.float32 scalar (shape () like the reference).
"""

import numpy as np

N = 8192
D = 256
NCORES = 8
RPC = N // NCORES          # 1024 rows per core
P = 128                    # partitions
NT_P = RPC // P            # 8 positive tiles / m-chunks
KH = D // P                # 2 k-halves
NB = 512                   # matmul free-dim per instruction (1 psum bank)
GCOLS = 2048               # columns per activation / column group
NGRP = N // GCOLS          # 4 column groups
TPG = GCOLS // P           # 16 anchor tiles per column group
HTPG = TPG // 2            # 8 tiles per group-0 half
NSLOT = NGRP + 1           # ssum slots per m-chunk (g0 split into 2)
EPS = 1e-8                 # reference eps (negligible for randn rows)

MM_DTYPE = "bfloat16"

_BUILD_CACHE = {}
_ACT_TABLES_PATCHED = False


def _patch_act_tables():
    """Make both Exp and Ln resolve to the one table set that contains
    them both (natural_log_exp_and_others), so the kernel needs a single
    ACT_TABLE_LOAD instead of thrashing between exp/ln sets.  Set ids
    are positional, so we filter set contents rather than reorder."""
    global _ACT_TABLES_PATCHED
    if _ACT_TABLES_PATCHED:
        return
    import concourse.bacc as bacc_mod
    import concourse.bass_interp as interp_mod
    import concourse.mybir as mybir
    from concourse import hw_specs

    AF = mybir.ActivationFunctionType
    orig = hw_specs.get_activation_tables

    def patched(module_arch):
        tabs = orig(module_arch)
        out = {}
        for name, funcs in tabs.items():
            f = set(funcs)
            if name != "natural_log_exp_and_others":
                f.discard(AF.Exp)
                f.discard(AF.Ln)
            out[name] = f
        return out

    bacc_mod.get_activation_tables = patched
    interp_mod.get_activation_tables = patched
    _ACT_TABLES_PATCHED = True


def _build(w: float, b: float, mm_dtype: str):
    from contextlib import ExitStack

    import concourse.bass as bass  # noqa: F401
    import concourse.mybir as mybir
    import concourse.tile as tile
    from concourse import bacc

    _patch_act_tables()

    f32 = mybir.dt.float32
    mm_dt = getattr(mybir.dt, mm_dtype)
    AF = mybir.ActivationFunctionType
    ALU = mybir.AluOpType
    AX = mybir.AxisListType

    absw = abs(float(w))
    bias_exp = -absw          # exp(scale_i * dot + b - shift), shift = |w| + b

    nc = bacc.Bacc("TRN2", target_bir_lowering=False, debug=False)

    xp = nc.dram_tensor("xp", [RPC, D], f32, kind="ExternalInput").ap()
    xa = nc.dram_tensor("xa", [N, D], f32, kind="ExternalInput").ap()
    out_partial = nc.dram_tensor("partial", [1, 1], f32, kind="ExternalOutput").ap()

    xa_tiled = xa.rearrange("(t p) d -> p t d", p=P)   # 64 tiles of 128 rows

    with tile.TileContext(nc) as tc:
        with ExitStack() as ctx:
            sing = ctx.enter_context(tc.tile_pool(name="sing", bufs=1))
            sq_pool = ctx.enter_context(tc.tile_pool(name="sqp", bufs=3))
            exp_pool = ctx.enter_context(tc.tile_pool(name="expp", bufs=3))

            # ---- persistent SBUF tensors (split per group for fine deps)
            xa_raw = [sing.tile([P, TPG * D], f32, tag=f"xar{g}", name=f"xar{g}")
                      for g in range(NGRP)]
            xa_bf = [sing.tile([P, TPG * D], mm_dt, tag=f"xab{g}", name=f"xab{g}")
                     for g in range(NGRP)]
            ssq_a = [sing.tile([P, TPG], f32, tag=f"ssqa{g}", name=f"ssqa{g}")
                     for g in range(NGRP)]
            lns_a = [sing.tile([P, TPG], f32, tag=f"lnsa{g}", name=f"lnsa{g}")
                     for g in range(NGRP)]
            inv_a = [sing.tile([P, TPG], f32, tag=f"inva{g}", name=f"inva{g}")
                     for g in range(NGRP)]
            # transposed anchors, h-major: [P, h, col]
            ant = [sing.tile([P, KH * GCOLS], mm_dt, tag=f"ant{g}",
                             name=f"ant{g}") for g in range(NGRP)]

            sb_xp = sing.tile([P, NT_P * D], f32, tag="xp")     # positives raw
            sb_xp_bf = sing.tile([P, NT_P * D], mm_dt, tag="xpbf")
            # transposed positives, m-major then h: [P, m, h, col]
            pnt = sing.tile([P, NT_P * KH * P], mm_dt, tag="pnt")
            ones = sing.tile([P, 1], f32, tag="ones")
            bias_t = sing.tile([P, 1], f32, tag="bias_t")

            ssq_p = sing.tile([P, NT_P], f32, tag="ssqp")
            lns_p = sing.tile([P, NT_P], f32, tag="lnsp")
            inv_p = sing.tile([P, NT_P], f32, tag="invp")
            winvp = sing.tile([P, NT_P], f32, tag="winvp")       # w / |p_i|
            pa = sing.tile([P, NT_P], f32, tag="pa")             # dot(p_i,a_i)
            ssum = sing.tile([P, NT_P * NSLOT], f32, tag="ssum")
            srow = sing.tile([P, NT_P], f32, tag="srow")
            lnS = sing.tile([P, NT_P], f32, tag="lnS")
            cosd = sing.tile([P, NT_P], f32, tag="cosd")
            rowloss = sing.tile([P, NT_P], f32, tag="rowloss")
            rsum = sing.tile([P, 1], f32, tag="rsum")
            sc_out = sing.tile([1, 1], f32, tag="sc_out")

            nc.vector.memset(ones, 1.0)
            nc.vector.memset(bias_t, bias_exp)

            # ---- loads: priority stream (ACT hwdge queues): xp, then the
            # two group-0 halves; bulk stream (gpsimd swdge queues): g1-3.
            # Each queue set is FIFO, so data arrives in issue order and
            # group-0 work starts ~12us in instead of after the full 9MB.
            nc.scalar.dma_start(
                out=sb_xp.rearrange("p (t d) -> p t d", d=D),
                in_=xp.rearrange("(t p) d -> p t d", p=P),
            )
            for half in range(2):
                nc.scalar.dma_start(
                    out=xa_raw[0].rearrange("p (t d) -> p t d", d=D)[
                        :, half * HTPG:(half + 1) * HTPG, :],
                    in_=xa_tiled[:, half * HTPG:(half + 1) * HTPG, :],
                )
            for g in range(1, NGRP):
                nc.gpsimd.dma_start(
                    out=xa_raw[g].rearrange("p (t d) -> p t d", d=D),
                    in_=xa_tiled[:, g * TPG:(g + 1) * TPG, :],
                )

            # ---- P-side prep ------------------------------------------
            def sumsq_act(src, t, acc, col):
                scr = sq_pool.tile([P, D], f32, tag="asqscr", name="asqscr")
                nc.scalar.activation(
                    scr, src[:, t * D:(t + 1) * D], AF.Square,
                    accum_out=acc[:, col:col + 1],
                )

            def sumsq_dve(src, t, acc, col):
                scr = sq_pool.tile([P, D], f32, tag="sqscr", name="sqscr")
                nc.vector.scalar_tensor_tensor(
                    out=scr,
                    in0=src[:, t * D:(t + 1) * D],
                    scalar=1.0,
                    in1=src[:, t * D:(t + 1) * D],
                    op0=ALU.mult,
                    op1=ALU.mult,
                    accum_out=acc[:, col:col + 1],
                )

            # xp: cast to bf16 on DVE, sumsq on ACT (idle this early)
            for half in range(2):
                nc.vector.tensor_copy(
                    sb_xp_bf[:, half * 4 * D:(half + 1) * 4 * D],
                    sb_xp[:, half * 4 * D:(half + 1) * 4 * D],
                )
            for t in range(NT_P):
                sumsq_act(sb_xp, t, ssq_p, t)
            nc.scalar.activation(lns_p, ssq_p, AF.Ln)
            nc.scalar.activation(inv_p, lns_p, AF.Exp, scale=-0.5)
            nc.vector.tensor_scalar_mul(winvp, inv_p, float(w))

            # positive transposes over the DMA xbar (sync hwdge queues)
            pnt4 = pnt.rearrange("p (m h c) -> p m h c", h=KH, c=P)
            for t in range(NT_P):
                for h in range(KH):
                    nc.sync.dma_start(
                        out=pnt4[:, t, h, :],
                        in_=sb_xp_bf[:, t * D + h * P: t * D + (h + 1) * P],
                        transpose=True,
                    )

            # ---- anchor prep per group --------------------------------
            def anchor_norms(g, t0, t1):
                nc.scalar.activation(lns_a[g][:, t0:t1], ssq_a[g][:, t0:t1],
                                     AF.Ln)
                nc.scalar.activation(inv_a[g][:, t0:t1], lns_a[g][:, t0:t1],
                                     AF.Exp, scale=-0.5)

            def anchor_prep(g, t0, t1):
                for t in range(t0, t1):
                    sumsq_dve(xa_raw[g], t, ssq_a[g], t)
                anchor_norms(g, t0, t1)
                for t in range(t0, t1):
                    nc.vector.tensor_scalar_mul(
                        xa_bf[g][:, t * D:(t + 1) * D],
                        xa_raw[g][:, t * D:(t + 1) * D],
                        inv_a[g][:, t:t + 1],
                    )
                for t in range(t0, t1):
                    for h in range(KH):
                        nc.sync.dma_start(
                            out=ant[g][:, h * GCOLS + t * P:
                                       h * GCOLS + (t + 1) * P],
                            in_=xa_bf[g][:, t * D + h * P: t * D + (h + 1) * P],
                            transpose=True,
                        )

            def mm_exp(g, m, c0, c1, slot):
                cols = c1 - c0
                ps = psM.tile([P, cols], f32, tag="psmm", name="psmm")
                for h in range(KH):
                    for i in range(cols // NB):
                        nc.tensor.matmul(
                            ps[:, i * NB:(i + 1) * NB],
                            pnt4[:, m, h, :],
                            ant[g][:, h * GCOLS + c0 + i * NB:
                                   h * GCOLS + c0 + (i + 1) * NB],
                            start=(h == 0),
                            stop=(h == KH - 1),
                        )
                scr = exp_pool.tile([P, GCOLS], f32, tag="expscr",
                                    name="expscr")
                nc.scalar.activation(
                    scr[:, 0:cols],
                    ps,
                    AF.Exp,
                    bias=bias_t[:, 0:1],
                    scale=winvp[:, m:m + 1],
                    accum_out=ssum[:, m * NSLOT + slot: m * NSLOT + slot + 1],
                )

            with tc.tile_pool(name="psM", bufs=2, space="PSUM") as psM:
                # group 0 in halves: exp stream starts after 1MB of anchors
                for half in range(2):
                    anchor_prep(0, half * HTPG, (half + 1) * HTPG)
                    for m in range(NT_P):
                        mm_exp(0, m, half * HTPG * P, (half + 1) * HTPG * P,
                               half)
                for g in range(1, NGRP):
                    anchor_prep(g, 0, TPG)
                    for m in range(NT_P):
                        mm_exp(g, m, 0, GCOLS, g + 1)

            # ---- diagonal (exact fp32): rows of xp vs first 8 anchor
            # tiles of the rotated xa (== this core's own anchors)
            for t in range(NT_P):
                scr = sq_pool.tile([P, D], f32, tag="sqscr", name="sqscr")
                nc.vector.scalar_tensor_tensor(
                    out=scr,
                    in0=sb_xp[:, t * D:(t + 1) * D],
                    scalar=1.0,
                    in1=xa_raw[0][:, t * D:(t + 1) * D],
                    op0=ALU.mult,
                    op1=ALU.mult,
                    accum_out=pa[:, t:t + 1],
                )

            # ---- tail --------------------------------------------------
            nc.vector.tensor_reduce(
                srow,
                ssum.rearrange("p (m g) -> p m g", g=NSLOT),
                axis=AX.X,
                op=ALU.add,
            )
            nc.scalar.activation(lnS, srow, AF.Ln)
            # rowloss = lnS + |w| - winvp*inv_a0*pa
            nc.vector.tensor_mul(cosd, pa, inv_a[0][:, 0:NT_P])
            nc.vector.tensor_mul(cosd, cosd, winvp)   # = w * cos_ii
            nc.vector.scalar_tensor_tensor(
                out=rowloss,
                in0=cosd,
                scalar=-1.0,
                in1=lnS,
                op0=ALU.mult,
                op1=ALU.add,
            )
            nc.vector.tensor_scalar_add(rowloss, rowloss, absw)
            nc.vector.reduce_sum(rsum, rowloss, axis=AX.X)

            with tc.tile_pool(name="psF", bufs=1, space="PSUM") as psF:
                pfin = psF.tile([1, 1], f32, tag="pfin")
                nc.tensor.matmul(pfin, rsum, ones, start=True, stop=True)
                nc.vector.tensor_copy(sc_out, pfin)
            nc.sync.dma_start(out=out_partial, in_=sc_out)

    nc.compile()
    return nc


def _get_nc(w: float, b: float):
    key = (float(w), float(b), MM_DTYPE)
    if key not in _BUILD_CACHE:
        _BUILD_CACHE[key] = _build(float(w), float(b), MM_DTYPE)
    return _BUILD_CACHE[key]


def kernel(x, w, b, epoch=None, **_unused):
    from concourse.bass_utils import run_bass_kernel_spmd

    x = np.asarray(x, dtype=np.float32)
    w_f = float(np.asarray(w))
    b_f = float(np.asarray(b))
    assert x.shape == (N, 2, D), x.shape

    nc = _get_nc(w_f, b_f)

    xa_full = np.ascontiguousarray(x[:, 1, :])
    in_maps = []
    for c in range(NCORES):
        r0 = c * RPC
        in_maps.append({
            "xp": np.ascontiguousarray(x[r0:r0 + RPC, 0, :]),
            "xa": np.ascontiguousarray(np.roll(xa_full, -r0, axis=0)),
        })

    res = run_bass_kernel_spmd(nc, in_maps, list(range(NCORES)))
    total = 0.0
    for c in range(NCORES):
        total += float(res.results[c]["partial"][0, 0])
    loss = total / N
    return np.float32(loss)


# revision 8
# speedup vs baseline: 1.8583x; 1.8583x over previous
"""Trainium2 Bass kernel for nn_LossFunction_12532714569881.

Computes, for x: [N=8192, 2, D=256] fp32, w, b scalars:
    P = x[:,0,:]; A = x[:,1,:]
    logits = (P @ A^T) / max(|p_i||a_j|, eps) * w + b        # [N, N]
    loss = -mean_i(log_softmax(logits)[i, i])

Strategy (8 NeuronCores, SPMD, single launch):
  - Row-shard the NxN logits: core c owns rows R=c*1024 .. R+1024.
  - Each core receives its positive block (xp) and the FULL anchor
    matrix ROTATED so its own 1024 anchors come first (xa_rot); the
    diagonal block is then always tiles 0..7 of group 0 -- one NEFF
    works for all cores and the separate diag-anchor load is gone.
  - Loads are PRIORITIZED: xp + anchor group 0 stream on the ACT hwdge
    queue set (issued first, in halves for early compute start); groups
    1-3 stream behind them on the gpsimd swdge queue set.  Compute on
    group g overlaps the loads of groups g+1..
  - Anchors: per-group sum-of-squares on DVE, 1/norm via
    exp(-0.5*ln(s)) on ACT (single activation table set via
    _patch_act_tables), normalize+bf16 cast on DVE, transposed on the
    tensor engine via identity matmuls into PSUM claims that share the
    matmul pool slots, copied back to SBUF on DVE.  (DMA xbar
    transposes were tried and are ~1.2us of engine time per 128x128
    block -- 13x the PE route.)
  - Positives stay raw: the per-row scale w/|p_i| folds into the exp
    activation's per-partition scale operand.
  - Main loop per column group: bf16 matmuls accumulate K=256 in two
    128-chunks into [128, 2048] PSUM tiles (2 bufs x 4 banks); the
    scalar engine applies exp(scale_i * dot - |w|) with a fused row-sum
    (accum_out).  Group 0 is processed in two 1024-column halves so the
    exp stream starts as soon as the first 1MB of anchors lands.
  - Since cos in [-1,1], logits <= |w|+b, so the constant shift |w|+b
    replaces the row-max pass of a standard softmax (no overflow).
  - The diagonal logit (the label term) is recomputed exactly in fp32
    on the vector engine from the raw blocks, so the bf16 matmul noise
    only perturbs the log-sum-exp, where it averages out.
  - Each core emits one partial scalar = sum of its 1024 row losses
    (row loss = ln(S_i) + |w| - w*cos_ii); the host sums 8 partials,
    divides by N.

kernel(**inputs) -> np.float32 scalar (shape () like the reference).
"""

import numpy as np

N = 8192
D = 256
NCORES = 8
RPC = N // NCORES          # 1024 rows per core
P = 128                    # partitions
NT_P = RPC // P            # 8 positive tiles / m-chunks
KH = D // P                # 2 k-halves
NB = 512                   # matmul free-dim per instruction (1 psum bank)
GCOLS = 2048               # columns per activation / column group
NGRP = N // GCOLS          # 4 column groups
TPG = GCOLS // P           # 16 anchor tiles per column group
HTPG = TPG // 2            # 8 tiles per group-0 half
NSLOT = NGRP + 1           # ssum slots per m-chunk (g0 split into 2)
EPS = 1e-8                 # reference eps (negligible for randn rows)

MM_DTYPE = "bfloat16"

_BUILD_CACHE = {}
_ACT_TABLES_PATCHED = False


def _patch_act_tables():
    """Make both Exp and Ln resolve to the one table set that contains
    them both (natural_log_exp_and_others), so the kernel needs a single
    ACT_TABLE_LOAD instead of thrashing between exp/ln sets.  Set ids
    are positional, so we filter set contents rather than reorder."""
    global _ACT_TABLES_PATCHED
    if _ACT_TABLES_PATCHED:
        return
    import concourse.bacc as bacc_mod
    import concourse.bass_interp as interp_mod
    import concourse.mybir as mybir
    from concourse import hw_specs

    AF = mybir.ActivationFunctionType
    orig = hw_specs.get_activation_tables

    def patched(module_arch):
        tabs = orig(module_arch)
        out = {}
        for name, funcs in tabs.items():
            f = set(funcs)
            if name != "natural_log_exp_and_others":
                f.discard(AF.Exp)
                f.discard(AF.Ln)
            out[name] = f
        return out

    bacc_mod.get_activation_tables = patched
    interp_mod.get_activation_tables = patched
    _ACT_TABLES_PATCHED = True


def _build(w: float, b: float, mm_dtype: str):
    from contextlib import ExitStack

    import concourse.bass as bass  # noqa: F401
    import concourse.mybir as mybir
    import concourse.tile as tile
    from concourse import bacc

    _patch_act_tables()

    f32 = mybir.dt.float32
    mm_dt = getattr(mybir.dt, mm_dtype)
    AF = mybir.ActivationFunctionType
    ALU = mybir.AluOpType
    AX = mybir.AxisListType

    absw = abs(float(w))
    bias_exp = -absw          # exp(scale_i * dot + b - shift), shift = |w| + b

    nc = bacc.Bacc("TRN2", target_bir_lowering=False, debug=False)

    xp = nc.dram_tensor("xp", [RPC, D], f32, kind="ExternalInput").ap()
    xa = nc.dram_tensor("xa", [N, D], f32, kind="ExternalInput").ap()
    out_partial = nc.dram_tensor("partial", [1, 1], f32, kind="ExternalOutput").ap()

    xa_tiled = xa.rearrange("(t p) d -> p t d", p=P)   # 64 tiles of 128 rows

    with tile.TileContext(nc) as tc:
        with ExitStack() as ctx:
            sing = ctx.enter_context(tc.tile_pool(name="sing", bufs=1))
            sq_pool = ctx.enter_context(tc.tile_pool(name="sqp", bufs=3))
            exp_pool = ctx.enter_context(tc.tile_pool(name="expp", bufs=3))

            # ---- persistent SBUF tensors (split per group for fine deps)
            xa_raw = [sing.tile([P, TPG * D], f32, tag=f"xar{g}", name=f"xar{g}")
                      for g in range(NGRP)]
            xa_bf = [sing.tile([P, TPG * D], mm_dt, tag=f"xab{g}", name=f"xab{g}")
                     for g in range(NGRP)]
            ssq_a = [sing.tile([P, TPG], f32, tag=f"ssqa{g}", name=f"ssqa{g}")
                     for g in range(NGRP)]
            lns_a = [sing.tile([P, TPG], f32, tag=f"lnsa{g}", name=f"lnsa{g}")
                     for g in range(NGRP)]
            inv_a = [sing.tile([P, TPG], f32, tag=f"inva{g}", name=f"inva{g}")
                     for g in range(NGRP)]
            # transposed anchors, h-major: [P, h, col]
            ant = [sing.tile([P, KH * GCOLS], mm_dt, tag=f"ant{g}",
                             name=f"ant{g}") for g in range(NGRP)]

            sb_xp = sing.tile([P, NT_P * D], f32, tag="xp")     # positives raw
            sb_xp_bf = sing.tile([P, NT_P * D], mm_dt, tag="xpbf")
            pnt = [sing.tile([P, RPC], mm_dt, tag=f"pnt{h}", name=f"pnt{h}")
                   for h in range(KH)]
            ident = sing.tile([P, P], mm_dt, tag="ident")
            ones = sing.tile([P, 1], f32, tag="ones")
            bias_t = sing.tile([P, 1], f32, tag="bias_t")

            ssq_p = sing.tile([P, NT_P], f32, tag="ssqp")
            lns_p = sing.tile([P, NT_P], f32, tag="lnsp")
            inv_p = sing.tile([P, NT_P], f32, tag="invp")
            winvp = sing.tile([P, NT_P], f32, tag="winvp")       # w / |p_i|
            pa = sing.tile([P, NT_P], f32, tag="pa")             # dot(p_i,a_i)
            ssum = sing.tile([P, NT_P * NSLOT], f32, tag="ssum")
            srow = sing.tile([P, NT_P], f32, tag="srow")
            lnS = sing.tile([P, NT_P], f32, tag="lnS")
            cosd = sing.tile([P, NT_P], f32, tag="cosd")
            rowloss = sing.tile([P, NT_P], f32, tag="rowloss")
            rsum = sing.tile([P, 1], f32, tag="rsum")
            sc_out = sing.tile([1, 1], f32, tag="sc_out")

            from concourse.masks import make_identity
            make_identity(nc, ident[:])
            nc.vector.memset(ones, 1.0)
            nc.vector.memset(bias_t, bias_exp)

            # ---- loads: priority stream (ACT hwdge queues): xp, then the
            # two group-0 halves; bulk stream (gpsimd swdge queues): g1-3.
            # Each queue set is FIFO, so data arrives in issue order and
            # group-0 work starts ~12us in instead of after the full 9MB.
            nc.scalar.dma_start(
                out=sb_xp.rearrange("p (t d) -> p t d", d=D),
                in_=xp.rearrange("(t p) d -> p t d", p=P),
            )
            for half in range(2):
                nc.scalar.dma_start(
                    out=xa_raw[0].rearrange("p (t d) -> p t d", d=D)[
                        :, half * HTPG:(half + 1) * HTPG, :],
                    in_=xa_tiled[:, half * HTPG:(half + 1) * HTPG, :],
                )
            for g in range(1, NGRP):
                nc.gpsimd.dma_start(
                    out=xa_raw[g].rearrange("p (t d) -> p t d", d=D),
                    in_=xa_tiled[:, g * TPG:(g + 1) * TPG, :],
                )

            # ---- P-side prep ------------------------------------------
            def sumsq_act(src, t, acc, col):
                scr = sq_pool.tile([P, D], f32, tag="asqscr", name="asqscr")
                nc.scalar.activation(
                    scr, src[:, t * D:(t + 1) * D], AF.Square,
                    accum_out=acc[:, col:col + 1],
                )

            def sumsq_dve(src, t, acc, col):
                scr = sq_pool.tile([P, D], f32, tag="sqscr", name="sqscr")
                nc.vector.scalar_tensor_tensor(
                    out=scr,
                    in0=src[:, t * D:(t + 1) * D],
                    scalar=1.0,
                    in1=src[:, t * D:(t + 1) * D],
                    op0=ALU.mult,
                    op1=ALU.mult,
                    accum_out=acc[:, col:col + 1],
                )

            # xp: cast to bf16 on DVE, sumsq on ACT (idle this early)
            for half in range(2):
                nc.vector.tensor_copy(
                    sb_xp_bf[:, half * 4 * D:(half + 1) * 4 * D],
                    sb_xp[:, half * 4 * D:(half + 1) * 4 * D],
                )
            for t in range(NT_P):
                sumsq_act(sb_xp, t, ssq_p, t)
            nc.scalar.activation(lns_p, ssq_p, AF.Ln)
            nc.scalar.activation(inv_p, lns_p, AF.Exp, scale=-0.5)
            nc.vector.tensor_scalar_mul(winvp, inv_p, float(w))

            # ---- anchor prep per group --------------------------------
            def anchor_norms(g, t0, t1):
                nc.scalar.activation(lns_a[g][:, t0:t1], ssq_a[g][:, t0:t1],
                                     AF.Ln)
                nc.scalar.activation(inv_a[g][:, t0:t1], lns_a[g][:, t0:t1],
                                     AF.Exp, scale=-0.5)

            with tc.tile_pool(name="psM", bufs=2, space="PSUM") as psM:
                def transpose_batch(src_bf, dst, h, t0, t1):
                    nt = t1 - t0
                    ps = psM.tile([P, nt * P], mm_dt, tag="psmm", name="pst")
                    for q in range(t0, t1):
                        nc.tensor.transpose(
                            ps[:, (q - t0) * P:(q - t0 + 1) * P],
                            src_bf[:, q * D + h * P: q * D + (h + 1) * P],
                            ident,
                        )
                    nc.vector.tensor_copy(dst, ps)

                # positive transposes (small, needed by every group)
                for h in range(KH):
                    transpose_batch(sb_xp_bf, pnt[h][:, :], h, 0, NT_P)

                def anchor_prep(g, t0, t1):
                    for t in range(t0, t1):
                        sumsq_dve(xa_raw[g], t, ssq_a[g], t)
                    anchor_norms(g, t0, t1)
                    for t in range(t0, t1):
                        nc.vector.tensor_scalar_mul(
                            xa_bf[g][:, t * D:(t + 1) * D],
                            xa_raw[g][:, t * D:(t + 1) * D],
                            inv_a[g][:, t:t + 1],
                        )
                    for h in range(KH):
                        transpose_batch(
                            xa_bf[g],
                            ant[g][:, h * GCOLS + t0 * P: h * GCOLS + t1 * P],
                            h, t0, t1,
                        )

                def mm_exp(g, m, c0, c1, slot):
                    cols = c1 - c0
                    ps = psM.tile([P, cols], f32, tag="psmm", name="psmm")
                    for h in range(KH):
                        for i in range(cols // NB):
                            nc.tensor.matmul(
                                ps[:, i * NB:(i + 1) * NB],
                                pnt[h][:, m * P:(m + 1) * P],
                                ant[g][:, h * GCOLS + c0 + i * NB:
                                       h * GCOLS + c0 + (i + 1) * NB],
                                start=(h == 0),
                                stop=(h == KH - 1),
                            )
                    scr = exp_pool.tile([P, GCOLS], f32, tag="expscr",
                                        name="expscr")
                    nc.scalar.activation(
                        scr[:, 0:cols],
                        ps,
                        AF.Exp,
                        bias=bias_t[:, 0:1],
                        scale=winvp[:, m:m + 1],
                        accum_out=ssum[:, m * NSLOT + slot:
                                       m * NSLOT + slot + 1],
                    )

                # group 0 in halves: exp stream starts after 1MB of anchors
                for half in range(2):
                    anchor_prep(0, half * HTPG, (half + 1) * HTPG)
                    for m in range(NT_P):
                        mm_exp(0, m, half * HTPG * P, (half + 1) * HTPG * P,
                               half)
                for g in range(1, NGRP):
                    anchor_prep(g, 0, TPG)
                    for m in range(NT_P):
                        mm_exp(g, m, 0, GCOLS, g + 1)

            # ---- diagonal (exact fp32): rows of xp vs first 8 anchor
            # tiles of the rotated xa (== this core's own anchors)
            for t in range(NT_P):
                scr = sq_pool.tile([P, D], f32, tag="sqscr", name="sqscr")
                nc.vector.scalar_tensor_tensor(
                    out=scr,
                    in0=sb_xp[:, t * D:(t + 1) * D],
                    scalar=1.0,
                    in1=xa_raw[0][:, t * D:(t + 1) * D],
                    op0=ALU.mult,
                    op1=ALU.mult,
                    accum_out=pa[:, t:t + 1],
                )

            # ---- tail --------------------------------------------------
            nc.vector.tensor_reduce(
                srow,
                ssum.rearrange("p (m g) -> p m g", g=NSLOT),
                axis=AX.X,
                op=ALU.add,
            )
            nc.scalar.activation(lnS, srow, AF.Ln)
            # rowloss = lnS + |w| - winvp*inv_a0*pa
            nc.vector.tensor_mul(cosd, pa, inv_a[0][:, 0:NT_P])
            nc.vector.tensor_mul(cosd, cosd, winvp)   # = w * cos_ii
            nc.vector.scalar_tensor_tensor(
                out=rowloss,
                in0=cosd,
                scalar=-1.0,
                in1=lnS,
                op0=ALU.mult,
                op1=ALU.add,
            )
            nc.vector.tensor_scalar_add(rowloss, rowloss, absw)
            nc.vector.reduce_sum(rsum, rowloss, axis=AX.X)

            with tc.tile_pool(name="psF", bufs=1, space="PSUM") as psF:
                pfin = psF.tile([1, 1], f32, tag="pfin")
                nc.tensor.matmul(pfin, rsum, ones, start=True, stop=True)
                nc.vector.tensor_copy(sc_out, pfin)
            nc.sync.dma_start(out=out_partial, in_=sc_out)

    nc.compile()
    return nc


def _get_nc(w: float, b: float):
    key = (float(w), float(b), MM_DTYPE)
    if key not in _BUILD_CACHE:
        _BUILD_CACHE[key] = _build(float(w), float(b), MM_DTYPE)
    return _BUILD_CACHE[key]


def kernel(x, w, b, epoch=None, **_unused):
    from concourse.bass_utils import run_bass_kernel_spmd

    x = np.asarray(x, dtype=np.float32)
    w_f = float(np.asarray(w))
    b_f = float(np.asarray(b))
    assert x.shape == (N, 2, D), x.shape

    nc = _get_nc(w_f, b_f)

    xa_full = np.ascontiguousarray(x[:, 1, :])
    in_maps = []
    for c in range(NCORES):
        r0 = c * RPC
        in_maps.append({
            "xp": np.ascontiguousarray(x[r0:r0 + RPC, 0, :]),
            "xa": np.ascontiguousarray(np.roll(xa_full, -r0, axis=0)),
        })

    res = run_bass_kernel_spmd(nc, in_maps, list(range(NCORES)))
    total = 0.0
    for c in range(NCORES):
        total += float(res.results[c]["partial"][0, 0])
    loss = total / N
    return np.float32(loss)


# revision 12
# speedup vs baseline: 2.1607x; 1.1627x over previous
"""Trainium2 Bass kernel for nn_LossFunction_12532714569881.

Computes, for x: [N=8192, 2, D=256] fp32, w, b scalars:
    P = x[:,0,:]; A = x[:,1,:]
    logits = (P @ A^T) / max(|p_i||a_j|, eps) * w + b        # [N, N]
    loss = -mean_i(log_softmax(logits)[i, i])

Strategy (8 NeuronCores, SPMD, single launch):
  - Row-shard the NxN logits: core c owns rows R=c*1024 .. R+1024.
  - Each core receives its positive block (xp) and the FULL anchor
    matrix ROTATED so its own 1024 anchors come first (xa_rot); the
    diagonal block is then always tiles 0..7 of group 0 -- one NEFF
    works for all cores and the separate diag-anchor load is gone.
  - Loads are PRIORITIZED: xp + anchor group 0 stream on the ACT hwdge
    queue set (issued first, in halves for early compute start); groups
    1-3 stream behind them on the gpsimd swdge queue set.  Compute on
    group g overlaps the loads of groups g+1..
  - Anchors: per-group sum-of-squares on DVE, 1/norm via
    exp(-0.5*ln(s)) on ACT (single activation table set via
    _patch_act_tables), normalize+bf16 cast on DVE, transposed on the
    tensor engine via identity matmuls into PSUM claims that share the
    matmul pool slots, copied back to SBUF on DVE.  (DMA xbar
    transposes were tried and are ~1.2us of engine time per 128x128
    block -- 13x the PE route.)
  - Positives stay raw: the per-row scale w/|p_i| folds into the exp
    activation's per-partition scale operand.
  - Main loop per column group: bf16 matmuls accumulate K=256 in two
    128-chunks into [128, 2048] PSUM tiles (2 bufs x 4 banks); the
    scalar engine applies exp(scale_i * dot - |w|) with a fused row-sum
    (accum_out).  Group 0 is processed in two 1024-column halves so the
    exp stream starts as soon as the first 1MB of anchors lands.
  - Since cos in [-1,1], logits <= |w|+b, so the constant shift |w|+b
    replaces the row-max pass of a standard softmax (no overflow).
  - The diagonal logit (the label term) is recomputed exactly in fp32
    on the vector engine from the raw blocks, so the bf16 matmul noise
    only perturbs the log-sum-exp, where it averages out.
  - Each core emits one partial scalar = sum of its 1024 row losses
    (row loss = ln(S_i) + |w| - w*cos_ii); the host sums 8 partials,
    divides by N.

kernel(**inputs) -> np.float32 scalar (shape () like the reference).
"""

import numpy as np

N = 8192
D = 256
NCORES = 8
RPC = N // NCORES          # 1024 rows per core
P = 128                    # partitions
NT_P = RPC // P            # 8 positive tiles / m-chunks
KH = D // P                # 2 k-halves
NB = 512                   # matmul free-dim per instruction (1 psum bank)
GCOLS = 2048               # columns per activation / column group
NGRP = N // GCOLS          # 4 column groups
TPG = GCOLS // P           # 16 anchor tiles per column group
HTPG = TPG // 2            # 8 tiles per group-0 half
NSLOT = NGRP + 1           # ssum slots per m-chunk (g0 split into 2)
EPS = 1e-8                 # reference eps (negligible for randn rows)

MM_DTYPE = "bfloat16"

_BUILD_CACHE = {}
_ACT_TABLES_PATCHED = False


def _patch_act_tables():
    """Make both Exp and Ln resolve to the one table set that contains
    them both (natural_log_exp_and_others), so the kernel needs a single
    ACT_TABLE_LOAD instead of thrashing between exp/ln sets.  Set ids
    are positional, so we filter set contents rather than reorder."""
    global _ACT_TABLES_PATCHED
    if _ACT_TABLES_PATCHED:
        return
    import concourse.bacc as bacc_mod
    import concourse.bass_interp as interp_mod
    import concourse.mybir as mybir
    from concourse import hw_specs

    AF = mybir.ActivationFunctionType
    orig = hw_specs.get_activation_tables

    def patched(module_arch):
        tabs = orig(module_arch)
        out = {}
        for name, funcs in tabs.items():
            f = set(funcs)
            if name != "natural_log_exp_and_others":
                f.discard(AF.Exp)
                f.discard(AF.Ln)
            out[name] = f
        return out

    bacc_mod.get_activation_tables = patched
    interp_mod.get_activation_tables = patched
    _ACT_TABLES_PATCHED = True


def _build(w: float, b: float, mm_dtype: str):
    from contextlib import ExitStack

    import concourse.bass as bass  # noqa: F401
    import concourse.mybir as mybir
    import concourse.tile as tile
    from concourse import bacc

    _patch_act_tables()

    f32 = mybir.dt.float32
    mm_dt = getattr(mybir.dt, mm_dtype)
    AF = mybir.ActivationFunctionType
    ALU = mybir.AluOpType
    AX = mybir.AxisListType

    absw = abs(float(w))
    bias_exp = -absw          # exp(scale_i * dot + b - shift), shift = |w| + b

    nc = bacc.Bacc("TRN2", target_bir_lowering=False, debug=False)

    xp = nc.dram_tensor("xp", [RPC, D], f32, kind="ExternalInput").ap()
    xa = nc.dram_tensor("xa", [N, D], f32, kind="ExternalInput").ap()
    out_partial = nc.dram_tensor("partial", [1, 1], f32, kind="ExternalOutput").ap()

    xa_tiled = xa.rearrange("(t p) d -> p t d", p=P)   # 64 tiles of 128 rows

    with tile.TileContext(nc) as tc:
        with ExitStack() as ctx:
            sing = ctx.enter_context(tc.tile_pool(name="sing", bufs=1))
            sq_pool = ctx.enter_context(tc.tile_pool(name="sqp", bufs=3))
            exp_pool = ctx.enter_context(tc.tile_pool(name="expp", bufs=3))

            # ---- persistent SBUF tensors (split per group for fine deps)
            xa_raw = [sing.tile([P, TPG * D], f32, tag=f"xar{g}", name=f"xar{g}")
                      for g in range(NGRP)]
            xa_bf = [sing.tile([P, TPG * D], mm_dt, tag=f"xab{g}", name=f"xab{g}")
                     for g in range(NGRP)]
            ssq_a = [sing.tile([P, TPG], f32, tag=f"ssqa{g}", name=f"ssqa{g}")
                     for g in range(NGRP)]
            lns_a = [sing.tile([P, TPG], f32, tag=f"lnsa{g}", name=f"lnsa{g}")
                     for g in range(NGRP)]
            inv_a = [sing.tile([P, TPG], f32, tag=f"inva{g}", name=f"inva{g}")
                     for g in range(NGRP)]
            # transposed anchors, h-major: [P, h, col]
            ant = [sing.tile([P, KH * GCOLS], mm_dt, tag=f"ant{g}",
                             name=f"ant{g}") for g in range(NGRP)]

            sb_xp = sing.tile([P, NT_P * D], f32, tag="xp")     # positives raw
            sb_xp_bf = sing.tile([P, NT_P * D], mm_dt, tag="xpbf")
            pnt = [sing.tile([P, RPC], mm_dt, tag=f"pnt{h}", name=f"pnt{h}")
                   for h in range(KH)]
            ident = sing.tile([P, P], mm_dt, tag="ident")
            ones = sing.tile([P, 1], f32, tag="ones")
            bias_t = sing.tile([P, 1], f32, tag="bias_t")

            ssq_p = sing.tile([P, NT_P], f32, tag="ssqp")
            lns_p = sing.tile([P, NT_P], f32, tag="lnsp")
            inv_p = sing.tile([P, NT_P], f32, tag="invp")
            winvp = sing.tile([P, NT_P], f32, tag="winvp")       # w / |p_i|
            pa = sing.tile([P, NT_P], f32, tag="pa")             # dot(p_i,a_i)
            ssum = sing.tile([P, NT_P * NSLOT], f32, tag="ssum")
            srow = sing.tile([P, NT_P], f32, tag="srow")
            lnS = sing.tile([P, NT_P], f32, tag="lnS")
            cosd = sing.tile([P, NT_P], f32, tag="cosd")
            rowloss = sing.tile([P, NT_P], f32, tag="rowloss")
            rsum = sing.tile([P, 1], f32, tag="rsum")
            sc_out = sing.tile([1, 1], f32, tag="sc_out")

            from concourse.masks import make_identity
            make_identity(nc, ident[:])
            nc.vector.memset(ones, 1.0)
            nc.vector.memset(bias_t, bias_exp)

            # ---- loads: ALL on the sync hwdge queue set, in priority
            # order: xp, group-0 halves, then groups 1-3.  One queue set
            # is FIFO at full DMA bandwidth, so data lands in exactly
            # this order (two concurrent queue sets split bandwidth and
            # starve the critical prefix -- measured).
            nc.sync.dma_start(
                out=sb_xp.rearrange("p (t d) -> p t d", d=D),
                in_=xp.rearrange("(t p) d -> p t d", p=P),
            )
            for half in range(2):
                nc.sync.dma_start(
                    out=xa_raw[0].rearrange("p (t d) -> p t d", d=D)[
                        :, half * HTPG:(half + 1) * HTPG, :],
                    in_=xa_tiled[:, half * HTPG:(half + 1) * HTPG, :],
                )
            for g in range(1, NGRP):
                nc.sync.dma_start(
                    out=xa_raw[g].rearrange("p (t d) -> p t d", d=D),
                    in_=xa_tiled[:, g * TPG:(g + 1) * TPG, :],
                )

            # ---- P-side prep ------------------------------------------
            def sumsq_dve(src, t, acc, col):
                scr = sq_pool.tile([P, D], f32, tag="sqscr", name="sqscr")
                nc.vector.scalar_tensor_tensor(
                    out=scr,
                    in0=src[:, t * D:(t + 1) * D],
                    scalar=1.0,
                    in1=src[:, t * D:(t + 1) * D],
                    op0=ALU.mult,
                    op1=ALU.mult,
                    accum_out=acc[:, col:col + 1],
                )

            # xp: cast to bf16 + sumsq on DVE (keeping ssq off ACT means
            # the scalar engine only ever needs the exp/ln table set --
            # a single ACT_TABLE_LOAD for the whole kernel)
            for half in range(2):
                nc.vector.tensor_copy(
                    sb_xp_bf[:, half * 4 * D:(half + 1) * 4 * D],
                    sb_xp[:, half * 4 * D:(half + 1) * 4 * D],
                )
            for t in range(NT_P):
                sumsq_dve(sb_xp, t, ssq_p, t)
            nc.scalar.activation(lns_p, ssq_p, AF.Ln)
            nc.scalar.activation(inv_p, lns_p, AF.Exp, scale=-0.5)
            nc.vector.tensor_scalar_mul(winvp, inv_p, float(w))

            # ---- anchor prep per group --------------------------------
            def anchor_norms(g, t0, t1):
                nc.scalar.activation(lns_a[g][:, t0:t1], ssq_a[g][:, t0:t1],
                                     AF.Ln)
                nc.scalar.activation(inv_a[g][:, t0:t1], lns_a[g][:, t0:t1],
                                     AF.Exp, scale=-0.5)

            with tc.tile_pool(name="psM", bufs=2, space="PSUM") as psM:
                u32 = mybir.dt.uint32

                def transpose_batch(src_bf, dst, h, t0, t1):
                    nt = t1 - t0
                    ps = psM.tile([P, nt * P], mm_dt, tag="psmm", name="pst")
                    for q in range(t0, t1):
                        nc.tensor.transpose(
                            ps[:, (q - t0) * P:(q - t0 + 1) * P],
                            src_bf[:, q * D + h * P: q * D + (h + 1) * P],
                            ident,
                        )
                    # copy psum->sbuf as packed u32 words (half the cols)
                    nc.vector.tensor_copy(dst.bitcast(u32), ps.bitcast(u32))

                # positive transposes (small, needed by every group)
                for h in range(KH):
                    transpose_batch(sb_xp_bf, pnt[h][:, :], h, 0, NT_P)

                def anchor_prep(g, t0, t1):
                    for t in range(t0, t1):
                        sumsq_dve(xa_raw[g], t, ssq_a[g], t)
                    anchor_norms(g, t0, t1)
                    for t in range(t0, t1):
                        nc.vector.tensor_scalar_mul(
                            xa_bf[g][:, t * D:(t + 1) * D],
                            xa_raw[g][:, t * D:(t + 1) * D],
                            inv_a[g][:, t:t + 1],
                        )
                    for h in range(KH):
                        transpose_batch(
                            xa_bf[g],
                            ant[g][:, h * GCOLS + t0 * P: h * GCOLS + t1 * P],
                            h, t0, t1,
                        )

                def mm_exp(g, m, c0, c1, slot):
                    cols = c1 - c0
                    ps = psM.tile([P, cols], f32, tag="psmm", name="psmm")
                    for h in range(KH):
                        for i in range(cols // NB):
                            nc.tensor.matmul(
                                ps[:, i * NB:(i + 1) * NB],
                                pnt[h][:, m * P:(m + 1) * P],
                                ant[g][:, h * GCOLS + c0 + i * NB:
                                       h * GCOLS + c0 + (i + 1) * NB],
                                start=(h == 0),
                                stop=(h == KH - 1),
                            )
                    scr = exp_pool.tile([P, GCOLS], f32, tag="expscr",
                                        name="expscr")
                    nc.scalar.activation(
                        scr[:, 0:cols],
                        ps,
                        AF.Exp,
                        bias=bias_t[:, 0:1],
                        scale=winvp[:, m:m + 1],
                        accum_out=ssum[:, m * NSLOT + slot:
                                       m * NSLOT + slot + 1],
                    )

                # group 0 in halves: exp stream starts after 1MB of anchors
                for half in range(2):
                    anchor_prep(0, half * HTPG, (half + 1) * HTPG)
                    for m in range(NT_P):
                        mm_exp(0, m, half * HTPG * P, (half + 1) * HTPG * P,
                               half)
                for g in range(1, NGRP):
                    anchor_prep(g, 0, TPG)
                    for m in range(NT_P):
                        mm_exp(g, m, 0, GCOLS, g + 1)

            # ---- diagonal (exact fp32): rows of xp vs first 8 anchor
            # tiles of the rotated xa (== this core's own anchors)
            for t in range(NT_P):
                scr = sq_pool.tile([P, D], f32, tag="sqscr", name="sqscr")
                nc.vector.scalar_tensor_tensor(
                    out=scr,
                    in0=sb_xp[:, t * D:(t + 1) * D],
                    scalar=1.0,
                    in1=xa_raw[0][:, t * D:(t + 1) * D],
                    op0=ALU.mult,
                    op1=ALU.mult,
                    accum_out=pa[:, t:t + 1],
                )

            # ---- tail --------------------------------------------------
            nc.vector.tensor_reduce(
                srow,
                ssum.rearrange("p (m g) -> p m g", g=NSLOT),
                axis=AX.X,
                op=ALU.add,
            )
            nc.scalar.activation(lnS, srow, AF.Ln)
            # rowloss = lnS + |w| - winvp*inv_a0*pa
            nc.vector.tensor_mul(cosd, pa, inv_a[0][:, 0:NT_P])
            nc.vector.tensor_mul(cosd, cosd, winvp)   # = w * cos_ii
            nc.vector.scalar_tensor_tensor(
                out=rowloss,
                in0=cosd,
                scalar=-1.0,
                in1=lnS,
                op0=ALU.mult,
                op1=ALU.add,
            )
            nc.vector.tensor_scalar_add(rowloss, rowloss, absw)
            nc.vector.reduce_sum(rsum, rowloss, axis=AX.X)

            with tc.tile_pool(name="psF", bufs=1, space="PSUM") as psF:
                pfin = psF.tile([1, 1], f32, tag="pfin")
                nc.tensor.matmul(pfin, rsum, ones, start=True, stop=True)
                nc.vector.tensor_copy(sc_out, pfin)
            nc.sync.dma_start(out=out_partial, in_=sc_out)

    nc.compile()
    return nc


def _get_nc(w: float, b: float):
    key = (float(w), float(b), MM_DTYPE)
    if key not in _BUILD_CACHE:
        _BUILD_CACHE[key] = _build(float(w), float(b), MM_DTYPE)
    return _BUILD_CACHE[key]


def kernel(x, w, b, epoch=None, **_unused):
    from concourse.bass_utils import run_bass_kernel_spmd

    x = np.asarray(x, dtype=np.float32)
    w_f = float(np.asarray(w))
    b_f = float(np.asarray(b))
    assert x.shape == (N, 2, D), x.shape

    nc = _get_nc(w_f, b_f)

    xa_full = np.ascontiguousarray(x[:, 1, :])
    in_maps = []
    for c in range(NCORES):
        r0 = c * RPC
        in_maps.append({
            "xp": np.ascontiguousarray(x[r0:r0 + RPC, 0, :]),
            "xa": np.ascontiguousarray(np.roll(xa_full, -r0, axis=0)),
        })

    res = run_bass_kernel_spmd(nc, in_maps, list(range(NCORES)))
    total = 0.0
    for c in range(NCORES):
        total += float(res.results[c]["partial"][0, 0])
    loss = total / N
    return np.float32(loss)
